# revision 28
# baseline (speedup 1.0000x reference)
"""APPNP GNN message passing on 8 Trainium2 NeuronCores.

Algorithm (u-space):  u_0 = dinv*h,  u_{k+1}[t] = a[t]*sum_{e->t} u_k[src] + g[t]
with a = 0.9*dinv^2, g = 0.1*dinv*h, h = relu(x@W1+b1), dinv = 1/sqrt(deg);
output z_K = 0.9*dinv*S_K + 0.1*h.

Sharding: 12500 target nodes per core. Each hop: per-chunk (4 x 25088-row
windows of the replicated node table) degree-sorted batched dma_gather of
source rows, DVE segmented reduce + per-target scale, dma_scatter_add of
partial sums into a g-initialized accumulator, AllGather to refresh every
core's table replica.
"""
import numpy as np

import concourse.bass as bass
import concourse.mybir as mybir
from concourse.bass_utils import run_bass_kernel_spmd
from concourse.library_config import mlp
from concourse.library_overlay import lower_extended_insts

# problem constants (hardcoded per task spec)
N = 100000
E = 1600000
IN_CH = 256
OUT_CH = 64
K = 10
ALPHA = 0.1

NCORES = 8
SHARD = 12500            # real nodes per core
SROWS = 12544            # stripe rows (= 98*128), rows 12500.. are zero pads
NB_LIN = SROWS // 128    # 98 lin1 batches
TROWS = NCORES * SROWS   # 100352 table rows
NCHUNKS = 4
CHUNK = TROWS // NCHUNKS  # 25088 (= 2 stripes, < 32768 so int16 indexes work)
ZROW = 12500             # per-chunk local row that is always zero
COLS_MAX = 64            # max gather-call columns (SBUF budget)
BUFS = 7                 # gather tile slots (>= NQ+1 keeps all queues busy)
NQ = 4                   # SWDGE queues; in-flight packets pipeline HBM latency
SINGLE_PACKET = False

f32 = mybir.dt.float32
i16 = mybir.dt.int16


def _wrap16(flat):
    """int16 list (len % 16 == 0) -> [128, len/16] wrapped + replicated x8."""
    L = len(flat) // 16
    a = flat.reshape(L, 16).T.astype(np.int16)   # [16, L]
    return np.tile(a, (8, 1))


def _srow(n):
    return (n // SHARD) * SROWS + (n % SHARD)


def build_plan(edge_index):
    """Host-side graph preprocessing. Returns global call structure +
    per-core input arrays.

    Self-loop edges are NOT placed in the gather streams; their term
    a[t]*u_k[t] (+ g[t]) is folded into the per-hop accumulator init
    (dst = a_no * u_own + g) computed by a DVE pass over the local
    stripe. This cuts gather tokens ~19% (direct tokens + the +1
    degree inflation of every self-chunk batch profile)."""
    row = np.asarray(edge_index[0], dtype=np.int64)
    col = np.asarray(edge_index[1], dtype=np.int64)

    # degree INCLUDES the self loop (reference semantics)
    deg = (np.bincount(col, minlength=N) + 1).astype(np.float64)
    dinv = (1.0 / np.sqrt(deg)).astype(np.float32)
    a_full = (0.9 * dinv * dinv).astype(np.float32)
    adr_full = (0.9 * dinv).astype(np.float32)

    srow_of = _srow(row)                 # table row of each edge's source
    chunk_of = srow_of // CHUNK
    local_of = (srow_of % CHUNK).astype(np.int64)
    core_of = col // SHARD
    t_local = (col % SHARD).astype(np.int64)

    # per (core, chunk): sorted targets and edge slots
    percore = [dict() for _ in range(NCORES)]
    nb_q = np.zeros(NCHUNKS, dtype=np.int64)
    d_global = [None] * NCHUNKS  # per chunk: [NBq] decreasing batch degrees

    # first pass: degree profiles
    d_sorted_all = [[None] * NCORES for _ in range(NCHUNKS)]
    order_all = [[None] * NCORES for _ in range(NCHUNKS)]
    edges_all = [[None] * NCORES for _ in range(NCHUNKS)]
    for c in range(NCORES):
        cm = core_of == c
        for q in range(NCHUNKS):
            m = cm & (chunk_of == q)
            t = t_local[m]
            s = local_of[m]
            d = np.bincount(t, minlength=SHARD)
            order = np.argsort(-d, kind="stable")
            d_sorted = d[order]
            d_sorted_all[q][c] = d_sorted
            order_all[q][c] = order
            edges_all[q][c] = (t, s)

    for q in range(NCHUNKS):
        counts = [int((ds > 0).sum()) for ds in d_sorted_all[q]]
        nb = (max(counts) + 127) // 128
        nb_q[q] = nb
        dg = np.zeros(nb, dtype=np.int64)
        for c in range(NCORES):
            ds = d_sorted_all[q][c]
            for b in range(nb):
                dg[b] = max(dg[b], ds[b * 128])
        assert dg.min() >= 1
        d_global[q] = dg

    # call structure: runs of equal D, capped at COLS_MAX columns
    calls = []  # (q, b0, nb, D)
    for q in range(NCHUNKS):
        dg = d_global[q]
        b = 0
        while b < len(dg):
            D = int(dg[b])
            b2 = b
            while b2 < len(dg) and dg[b2] == D and (b2 - b + 1) * D <= COLS_MAX:
                b2 += 1
            calls.append((q, b, b2 - b, D))
            b = b2

    nbtot = int(nb_q.sum())

    # per-core arrays
    core_inputs = []
    for c in range(NCORES):
        gather_parts = []
        a_sc = np.zeros((128, nbtot), np.float32)
        adr_sc = np.zeros((128, nbtot), np.float32)
        # cumulative batch column per (q, b)
        qbase = np.concatenate([[0], np.cumsum(nb_q)])[:NCHUNKS]

        # per chunk: slot grid [NBq*128, Dmax-ish] built per call
        for q in range(NCHUNKS):
            t, s = edges_all[q][c]
            order = order_all[q][c]
            nb = int(nb_q[q])
            rank_of = np.full(SHARD, -1, np.int64)
            rank_of[order] = np.arange(SHARD)
            r = rank_of[t]                      # slot row rank per edge
            # j = occurrence index of each edge within its target
            es = np.argsort(r, kind="stable")
            r_s = r[es]
            s_s = s[es]
            starts = np.searchsorted(r_s, np.arange(SHARD))
            j_s = np.arange(len(r_s)) - starts[r_s]
            # fill per-target padded grid lazily per call below
            grid = {}
            percore[c][q] = (r_s, j_s, s_s)

            # a tables
            d_sorted = d_sorted_all[q][c]
            for b in range(nb):
                tgt_rank = b * 128 + np.arange(128)
                valid = tgt_rank < SHARD
                tgt = order[np.minimum(tgt_rank, SHARD - 1)]
                valid &= d_sorted[np.minimum(tgt_rank, SHARD - 1)] > 0
                gnode = c * SHARD + tgt
                a_sc[:, qbase[q] + b] = np.where(valid, a_full[gnode], 0.0)
                adr_sc[:, qbase[q] + b] = np.where(valid, adr_full[gnode], 0.0)

        # gather index stream per call
        for (q, b0, nb, D) in calls:
            r_s, j_s, s_s = percore[c][q]
            cols = nb * D
            nidx = cols * 128
            flat = np.full(nidx, ZROW, np.int64)
            lo, hi = np.searchsorted(r_s, [b0 * 128, (b0 + nb) * 128])
            rr = r_s[lo:hi] - b0 * 128
            jj = j_s[lo:hi]
            ss = s_s[lo:hi]
            keep = jj < D  # should always hold (D >= batch max degree)
            rr, jj, ss = rr[keep], jj[keep], ss[keep]
            b_loc = rr // 128
            p = rr % 128
            colidx = b_loc * D + jj
            flat[colidx * 128 + p] = ss
            gather_parts.append(_wrap16(flat))
        gidx = np.concatenate(gather_parts, axis=1)

        # scatter rows per chunk
        sidx_parts = []
        for q in range(NCHUNKS):
            nb = int(nb_q[q])
            order = order_all[q][c]
            d_sorted = d_sorted_all[q][c]
            tgt_rank = np.arange(nb * 128)
            valid = (tgt_rank < SHARD)
            tgt = order[np.minimum(tgt_rank, SHARD - 1)]
            valid &= d_sorted[np.minimum(tgt_rank, SHARD - 1)] > 0
            rows = np.where(valid, tgt, ZROW)
            sidx_parts.append(_wrap16(rows))
        sidx = np.concatenate(sidx_parts, axis=1)

        # node-order dinv for lin1 scaling [128, NB_LIN]
        dinv_no = np.zeros((128, NB_LIN), np.float32)
        nodes = c * SHARD + np.arange(SHARD)
        dv = dinv[nodes]
        dinv_no.T.flat[:SHARD] = dv  # [b, p] row-major = node order
        # node-order a / adr for the self-loop fold pass
        a_no = np.zeros((128, NB_LIN), np.float32)
        a_no.T.flat[:SHARD] = a_full[nodes]
        adr_no = np.zeros((128, NB_LIN), np.float32)
        adr_no.T.flat[:SHARD] = adr_full[nodes]
        core_inputs.append(dict(gidx=gidx, sidx=sidx, a_sc=a_sc,
                                adr_sc=adr_sc, dinv_no=dinv_no,
                                a_no=a_no, adr_no=adr_no))

    plan = dict(calls=calls, nb_q=[int(x) for x in nb_q], nbtot=nbtot,
                gidx_cols=core_inputs[0]["gidx"].shape[1],
                sidx_cols=core_inputs[0]["sidx"].shape[1],
                core_inputs=core_inputs)
    return plan


def build_nc(plan, k_hops=K, stage=3):
    calls = plan["calls"]
    nb_q = plan["nb_q"]
    nbtot = plan["nbtot"]
    LG = plan["gidx_cols"]
    LS = plan["sidx_cols"]
    nbmax = max(nb_q)
    ncalls = len(calls)

    nc = bass.Bass(num_swdge_queues=NQ)
    xT = nc.declare_dram_parameter("xT", [IN_CH, SROWS], f32, isOutput=False)
    W1p = nc.declare_dram_parameter("W1", [IN_CH, OUT_CH], f32, isOutput=False)
    b1p = nc.declare_dram_parameter("b1", [1, OUT_CH], f32, isOutput=False)
    onesp = nc.declare_dram_parameter("ones", [1, 128], f32, isOutput=False)
    gidxp = nc.declare_dram_parameter("gidx", [128, LG], i16, isOutput=False)
    sidxp = nc.declare_dram_parameter("sidx", [128, LS], i16, isOutput=False)
    ap = nc.declare_dram_parameter("a_sc", [128, nbtot], f32, isOutput=False)
    adrp = nc.declare_dram_parameter("adr_sc", [128, nbtot], f32, isOutput=False)
    dinvp = nc.declare_dram_parameter("dinv_no", [128, NB_LIN], f32, isOutput=False)
    anop = nc.declare_dram_parameter("a_no", [128, NB_LIN], f32, isOutput=False)
    adrnop = nc.declare_dram_parameter("adr_no", [128, NB_LIN], f32, isOutput=False)
    out_t = nc.declare_dram_parameter("out", [SROWS, OUT_CH], f32, isOutput=True)

    utable = nc.dram_tensor("utable", [TROWS, OUT_CH], f32, addr_space="Shared")
    sbuf_b = nc.dram_tensor("sbufb", [SROWS, OUT_CH], f32)   # AllGather input
    g_dram = nc.dram_tensor("g_dram", [SROWS, OUT_CH], f32)
    gdr_dram = nc.dram_tensor("gdr_dram", [SROWS, OUT_CH], f32)

    NIN = 10  # sync-engine resident input loads

    # self-loop fold pass: per hop, dst_init = a_no * u_own + g, tiled
    # over row-groups of 8 lin1 batches (1024 rows); last group = 2.
    PASS_ITERS = []
    r = 0
    while r < SROWS:
        nb8 = min(3, (SROWS - r) // 128)
        PASS_ITERS.append((r, nb8))
        r += nb8 * 128
    NPASS = len(PASS_ITERS)

    from contextlib import ExitStack
    with ExitStack() as ctx:
        block = ctx.enter_context(nc.Block())
        sem_in = ctx.enter_context(nc.semaphore("sem_in"))
        # per-slot DMA sems: an aggregate count is only race-free when no
        # later DMA on the same sem is issued before the wait; slotted sems
        # keep at most one outstanding DMA per sem.
        sem_x = [ctx.enter_context(nc.semaphore(f"sem_x{i}")) for i in range(3)]
        sem_mm = ctx.enter_context(nc.semaphore("sem_mm"))
        sem_act = ctx.enter_context(nc.semaphore("sem_act"))
        sem_io = [ctx.enter_context(nc.semaphore(f"sem_io{i}"))
                  for i in range(4)]
        sem_p = ctx.enter_context(nc.semaphore("sem_p"))
        sem_cc = ctx.enter_context(nc.semaphore("sem_cc"))
        # per-slot pass sems: aggregate counts race with out-of-order DMA
        # completion when >1 DMA is in flight on one sem
        sem_pl = [ctx.enter_context(nc.semaphore(f"sem_pl{i}"))
                  for i in range(2)]
        sem_pv = ctx.enter_context(nc.semaphore("sem_pv"))
        sem_m = ctx.enter_context(nc.semaphore("sem_m"))
        sem_ps = [ctx.enter_context(nc.semaphore(f"sem_ps{i}"))
                  for i in range(2)]
        sem_g = [ctx.enter_context(nc.semaphore(f"sem_g{i}"))
                 for i in range(BUFS)]
        sem_r = ctx.enter_context(nc.semaphore("sem_r"))
        sem_s = ctx.enter_context(nc.semaphore("sem_s"))

        gidx_res = ctx.enter_context(nc.sbuf_tensor("gidx_res", [128, LG], i16))
        sidx_res = ctx.enter_context(nc.sbuf_tensor("sidx_res", [128, LS], i16))
        a_res = ctx.enter_context(nc.sbuf_tensor("a_res", [128, nbtot], f32))
        adr_res = ctx.enter_context(nc.sbuf_tensor("adr_res", [128, nbtot], f32))
        dinv_res = ctx.enter_context(nc.sbuf_tensor("dinv_res", [128, NB_LIN], f32))
        ano_res = ctx.enter_context(nc.sbuf_tensor("ano_res", [128, NB_LIN], f32))
        adrno_res = ctx.enter_context(
            nc.sbuf_tensor("adrno_res", [128, NB_LIN], f32))
        uin = ctx.enter_context(nc.sbuf_tensor("uin", [128, 2, 3, OUT_CH], f32))
        gin = ctx.enter_context(nc.sbuf_tensor("gin", [128, 2, 3, OUT_CH], f32))
        W1_sb = ctx.enter_context(nc.sbuf_tensor("W1_sb", [128, 2, OUT_CH], f32))
        b1_sb = ctx.enter_context(nc.sbuf_tensor("b1_sb", [1, OUT_CH], f32))
        ones_sb = ctx.enter_context(nc.sbuf_tensor("ones_sb", [1, 128], f32))
        xk = ctx.enter_context(nc.sbuf_tensor("xk", [128, 3, 2, 128], f32))
        h_sb = ctx.enter_context(nc.sbuf_tensor("h_sb", [128, 4, OUT_CH], f32))
        u0_sb = ctx.enter_context(nc.sbuf_tensor("u0_sb", [128, 4, OUT_CH], f32))
        g_sb = ctx.enter_context(nc.sbuf_tensor("g_sb", [128, 4, OUT_CH], f32))
        gdr_sb = ctx.enter_context(nc.sbuf_tensor("gdr_sb", [128, 4, OUT_CH], f32))
        z44 = ctx.enter_context(nc.sbuf_tensor("z44", [44, OUT_CH], f32))
        sparts = ctx.enter_context(
            nc.sbuf_tensor("sparts", [128, 2, nbmax, OUT_CH], f32))
        gt = ctx.enter_context(
            nc.sbuf_tensor("gt", [128, BUFS, COLS_MAX, OUT_CH], f32))
        psums = [ctx.enter_context(
            nc.psum_tensor(f"psum{i}", [128, OUT_CH], f32))
            for i in range(4)]

        # gather call offsets in gidx (in L-columns)
        goffs = []
        off = 0
        for (q, b0, nb, D) in calls:
            goffs.append(off)
            off += nb * D * 128 // 16
        assert off == LG
        soffs = []
        off = 0
        for q in range(NCHUNKS):
            soffs.append(off)
            off += nb_q[q] * 128 // 16
        assert off == LS
        qb = [0] * NCHUNKS
        acc = 0
        for q in range(NCHUNKS):
            qb[q] = acc
            acc += nb_q[q]

        @block.sync
        def _(sync):
            sync.dma_start(out=gidx_res[:], in_=gidxp[:]).then_inc(sem_in, 16)
            sync.dma_start(out=sidx_res[:], in_=sidxp[:]).then_inc(sem_in, 16)
            sync.dma_start(out=a_res[:], in_=ap[:]).then_inc(sem_in, 16)
            sync.dma_start(out=adr_res[:], in_=adrp[:]).then_inc(sem_in, 16)
            sync.dma_start(out=dinv_res[:], in_=dinvp[:]).then_inc(sem_in, 16)
            sync.dma_start(out=ano_res[:], in_=anop[:]).then_inc(sem_in, 16)
            sync.dma_start(out=adrno_res[:], in_=adrnop[:]).then_inc(sem_in, 16)
            sync.dma_start(
                out=W1_sb[:],
                in_=W1p[:].rearrange("(two p) c -> p two c", p=128),
            ).then_inc(sem_in, 16)
            sync.dma_start(out=b1_sb[:], in_=b1p[:]).then_inc(sem_in, 16)
            sync.dma_start(out=ones_sb[:], in_=onesp[:]).then_inc(sem_in, 16)
            for b in range(NB_LIN):
                if b >= 3:
                    sync.wait_ge(sem_mm, b - 2)
                sync.dma_start(
                    out=xk[:, b % 3, :, :],
                    in_=xT[:, b * 128:(b + 1) * 128].rearrange(
                        "(two p) n -> p two n", p=128),
                ).then_inc(sem_x[b % 3], 16)

        @block.tensor
        def _(tensor):
            tensor.wait_ge(sem_in, NIN * 16)
            for b in range(NB_LIN):
                tensor.wait_ge(sem_x[b % 3], 16 * (b // 3 + 1))
                if b >= 4:
                    tensor.wait_ge(sem_act, b - 3)
                ps = psums[b % 4]
                nc.tensor.matmul(ps[:], lhsT=xk[:, b % 3, 0, :],
                                 rhs=W1_sb[:, 0, :], start=True, stop=False)
                nc.tensor.matmul(ps[:], lhsT=xk[:, b % 3, 1, :],
                                 rhs=W1_sb[:, 1, :], start=False, stop=False)
                nc.tensor.matmul(ps[:], lhsT=ones_sb[:1, :],
                                 rhs=b1_sb[:1, :], start=False,
                                 stop=True).then_inc(sem_mm, 1)

        @block.scalar
        def _(scalar):
            scalar.wait_ge(sem_in, NIN * 16)
            AF = mybir.ActivationFunctionType
            for b in range(NB_LIN):
                scalar.wait_ge(sem_mm, b + 1)
                sl = b % 4
                if b >= 4:
                    scalar.wait_ge(sem_io[sl], 48 * (b // 4))
                nc.scalar.activation(h_sb[:, sl, :], psums[b % 4][:],
                                     AF.Relu).then_inc(sem_act, 1)
                nc.scalar.activation(u0_sb[:, sl, :], h_sb[:, sl, :], AF.Copy,
                                     scale=dinv_res[:, b:b + 1])
                nc.scalar.mul(g_sb[:, sl, :], u0_sb[:, sl, :], 0.1)
                nc.scalar.mul(gdr_sb[:, sl, :], h_sb[:, sl, :], 0.1)
                r0, r1 = b * 128, (b + 1) * 128
                nc.scalar.dma_start(out=sbuf_b[r0:r1, :],
                                    in_=u0_sb[:, sl, :]).then_inc(sem_io[sl], 16)
                nc.scalar.dma_start(out=g_dram[r0:r1, :],
                                    in_=g_sb[:, sl, :]).then_inc(sem_io[sl], 16)
                nc.scalar.dma_start(out=gdr_dram[r0:r1, :],
                                    in_=gdr_sb[:, sl, :]).then_inc(sem_io[sl], 16)
            # per-hop self-loop fold pass: dst = a_no*u_own + g (tiled,
            # store deferred one iter for load/compute overlap; flushed
            # before the hop boundary to avoid a cycle through sem_cc)
            pidx = 0
            mcall = 0
            lcount = {0: 0, 1: 0}
            scount = {0: 0, 1: 0}
            for k in range(k_hops):
                scalar.wait_ge(sem_cc, k + 1)
                dst = sbuf_b if k < k_hops - 1 else out_t
                gsrc = g_dram if k < k_hops - 1 else gdr_dram
                pending = None
                for (r0, nb8) in PASS_ITERS:
                    sl = pidx % 2
                    if pending is not None:
                        pr0, pnb8, psl = pending
                        scalar.wait_ge(sem_pv, pidx)
                        scount[psl] += 1
                        scalar.dma_start(
                            out=dst[pr0:pr0 + pnb8 * 128, :].rearrange(
                                "(b p) ch -> p b ch", p=128),
                            in_=uin[:, psl, :pnb8, :]).then_inc(
                                sem_ps[psl], 16)
                    if pidx >= 2:
                        # slot free when its previous store completed
                        scalar.wait_ge(sem_ps[sl], 16 * (pidx // 2))
                    rows = nb8 * 128
                    lcount[sl] += 1
                    scalar.dma_start(
                        out=uin[:, sl, :nb8, :],
                        in_=sbuf_b[r0:r0 + rows, :].rearrange(
                            "(b p) ch -> p b ch", p=128)).then_inc(
                                sem_pl[sl], 16)
                    scalar.dma_start(
                        out=gin[:, sl, :nb8, :],
                        in_=gsrc[r0:r0 + rows, :].rearrange(
                            "(b p) ch -> p b ch", p=128)).then_inc(
                                sem_pl[sl], 16)
                    pending = (r0, nb8, sl)
                    pidx += 1
                pr0, pnb8, psl = pending
                scalar.wait_ge(sem_pv, pidx)
                scount[psl] += 1
                scalar.dma_start(
                    out=dst[pr0:pr0 + pnb8 * 128, :].rearrange(
                        "(b p) ch -> p b ch", p=128),
                    in_=uin[:, psl, :pnb8, :]).then_inc(sem_ps[psl], 16)
                # per-target scale of partials (a or adr), one [128,1,64]
                # activation per batch with a per-partition scale vector
                tab_sc = a_res if k < k_hops - 1 else adr_res
                for q in range(NCHUNKS):
                    for ci, (cq, b0, nb, D) in enumerate(calls):
                        if cq != q:
                            continue
                        mcall += 1
                        scalar.wait_ge(sem_r, mcall)
                        for b in range(nb):
                            col = qb[q] + b0 + b
                            op = nc.scalar.mul(
                                sparts[:, q % 2, b0 + b:b0 + b + 1, :],
                                sparts[:, q % 2, b0 + b:b0 + b + 1, :],
                                tab_sc[:, col:col + 1])
                        op.then_inc(sem_m, 1)

        @block.vector
        def _(vector):
            if stage < 1:
                return
            vector.wait_ge(sem_in, NIN * 16)
            G = 0
            ppidx = 0
            for k in range(k_hops):
                tab = adr_res if k == k_hops - 1 else a_res
                ano = ano_res if k < k_hops - 1 else adrno_res
                for q in range(NCHUNKS):
                    if q == 1:
                        # self-loop fold compute (uin = a_no*uin + g),
                        # placed after chunk-0 reduces so the first
                        # reduce isn't delayed behind 13 pass ops
                        # (that stalled the Q7 gather pipeline at
                        # every hop start). Stores gate only the first
                        # scatter, which runs later than this.
                        for (r0, nb8) in PASS_ITERS:
                            sl = ppidx % 2
                            b0 = r0 // 128
                            vector.wait_ge(sem_pl[sl],
                                           32 * (ppidx // 2 + 1))
                            ab = ano[:, b0:b0 + nb8].rearrange(
                                "p b -> p b ()").broadcast_to(
                                    [128, nb8, OUT_CH])
                            nc.vector.tensor_tensor(
                                out=uin[:, sl, :nb8, :],
                                in0=uin[:, sl, :nb8, :],
                                in1=ab, op=mybir.AluOpType.mult)
                            nc.vector.tensor_tensor(
                                out=uin[:, sl, :nb8, :],
                                in0=uin[:, sl, :nb8, :],
                                in1=gin[:, sl, :nb8, :],
                                op=mybir.AluOpType.add).then_inc(sem_pv, 1)
                            ppidx += 1
                    if stage >= 2:
                        s_per_q = [(nb + 31) // 32 for nb in nb_q]
                        done_c = k * NCHUNKS + q - 2  # chunks fully scattered
                        if done_c >= 0:
                            done = sum(s_per_q[qq % NCHUNKS]
                                       for qq in range(done_c + 1))
                            vector.wait_ge(sem_s, 16 * done)
                    for ci, (cq, b0, nb, D) in enumerate(calls):
                        if cq != q:
                            continue
                        vector.wait_ge(sem_g[G % BUFS], 16 * (G // BUFS + 1))
                        cols = nb * D
                        seg = gt[:, G % BUFS, :cols, :].rearrange(
                            "p (b j) ch -> p b ch j", j=D)
                        # per-target a-scale moved to the idle ACT engine:
                        # sem_r fires right after the reduce, freeing the
                        # gather slot (Q7 backpressure) sooner
                        nc.vector.reduce_sum(
                            out=sparts[:, q % 2, b0:b0 + nb, :],
                            in_=seg,
                            axis=mybir.AxisListType.X).then_inc(sem_r, 1)
                        G += 1

        @block.gpsimd
        def _(gpsimd):
            gpsimd.load_library(mlp)
            nreg = nc.gpsimd.alloc_register("nreg")
            gpsimd.memset(z44[:], 0.0)
            gpsimd.dma_start(out=sbuf_b[SHARD:SROWS, :],
                             in_=z44[:]).then_inc(sem_p, 16)
            gpsimd.dma_start(out=g_dram[SHARD:SROWS, :],
                             in_=z44[:]).then_inc(sem_p, 16)
            gpsimd.wait_ge(sem_p, 32)
            for sl in range(4):
                uses = (NB_LIN - sl + 3) // 4  # batches with b%4==sl
                gpsimd.wait_ge(sem_io[sl], 48 * uses)
            gpsimd.collective_compute(
                "AllGather", mybir.AluOpType.bypass,
                ins=[sbuf_b[:]], outs=[utable[:]],
                replica_groups=[list(range(NCORES))],
            ).then_inc(sem_cc, 1)
            G = 0
            s_cnt = 0
            for k in range(k_hops):
                if stage < 1:
                    break
                gpsimd.wait_ge(sem_cc, k + 1)
                dst = sbuf_b if k < k_hops - 1 else out_t
                Gq = [0] * (NCHUNKS + 1)

                def scatter_chunk(q, s_cnt):
                    # serialize chunk scatters: same target rows, CCE RMW
                    gpsimd.wait_ge(sem_m, Gq[q + 1])
                    gpsimd.wait_ge(sem_s, 16 * s_cnt)
                    nb = nb_q[q]
                    # sub-calls of <=32 batches (4096 idx HW cap); rows
                    # unique within a chunk so sub-calls may overlap
                    for sb in range(0, nb, 32):
                        nbs = min(32, nb - sb)
                        nidx = nbs * 128
                        gpsimd.reg_mov(nreg, nidx)
                        gpsimd.dma_scatter_add(
                            dst[:], sparts[:, q % 2, sb:sb + nbs, :],
                            sidx_res[:, soffs[q] + sb * 8:
                                     soffs[q] + sb * 8 + nidx // 16],
                            nidx, nreg, OUT_CH,
                            single_packet=False,
                            queue_num=s_cnt % NQ,
                        ).then_inc(sem_s, 16)
                        s_cnt += 1
                    return s_cnt

                for q in range(NCHUNKS):
                    for ci, (cq, b0, nb, D) in enumerate(calls):
                        if cq != q:
                            continue
                        if G >= BUFS:
                            gpsimd.wait_ge(sem_r, G - BUFS + 1)
                        cols = nb * D
                        nidx = cols * 128
                        gpsimd.reg_mov(nreg, nidx)
                        gpsimd.dma_gather(
                            gt[:, G % BUFS, :cols, :],
                            utable[q * CHUNK:(q + 1) * CHUNK, :],
                            gidx_res[:, goffs[ci]:goffs[ci] + nidx // 16],
                            nidx, nreg, OUT_CH,
                            single_packet=SINGLE_PACKET,
                            queue_num=G % NQ,
                        ).then_inc(sem_g[G % BUFS], 16)
                        G += 1
                    Gq[q + 1] = G
                    # one-chunk lookahead: scatter chunk q-1 after chunk q's
                    # gathers are issued (keeps the gather stream fed while
                    # avoiding the reduce->scatter->issue cycle)
                    if stage >= 2 and q >= 1:
                        if q == 1:
                            tot = NPASS * (k + 1)
                            gpsimd.wait_ge(sem_ps[0], 16 * ((tot + 1) // 2))
                            gpsimd.wait_ge(sem_ps[1], 16 * (tot // 2))
                        s_cnt = scatter_chunk(q - 1, s_cnt)
                if stage >= 2:
                    s_cnt = scatter_chunk(NCHUNKS - 1, s_cnt)
                    gpsimd.wait_ge(sem_s, 16 * s_cnt)
                if stage >= 2 and k < k_hops - 1:
                    gpsimd.wait_ge(sem_s, 16 * s_cnt)
                    gpsimd.collective_compute(
                        "AllGather", mybir.AluOpType.bypass,
                        ins=[sbuf_b[:]], outs=[utable[:]],
                        replica_groups=[list(range(NCORES))],
                    ).then_inc(sem_cc, 1)
            if stage >= 2:
                gpsimd.wait_ge(sem_s, 16 * s_cnt)

    lower_extended_insts(nc)
    return nc


_CACHE = {}
TRACE = False          # test-harness knob; harness never sets it
LAST_RESULTS = None    # BassKernelResults of the last run when TRACE


def kernel(x, edge_index, W1, b1):
    x = np.asarray(x, dtype=np.float32)
    edge_index = np.asarray(edge_index)
    W1 = np.asarray(W1, dtype=np.float32)
    b1 = np.asarray(b1, dtype=np.float32)

    key = hash(edge_index[:, ::997].tobytes())
    if key in _CACHE:
        nc, plan = _CACHE[key]
    else:
        plan = build_plan(edge_index)
        nc = build_nc(plan)
        _CACHE[key] = (nc, plan)

    ones = np.ones((1, 128), np.float32)
    b1r = b1.reshape(1, OUT_CH)
    in_maps = []
    for c in range(NCORES):
        ci = plan["core_inputs"][c]
        xs = np.zeros((IN_CH, SROWS), np.float32)
        xs[:, :SHARD] = x[c * SHARD:(c + 1) * SHARD].T
        in_maps.append({
            "xT": np.ascontiguousarray(xs),
            "W1": W1, "b1": b1r, "ones": ones,
            "gidx": ci["gidx"], "sidx": ci["sidx"],
            "a_sc": ci["a_sc"], "adr_sc": ci["adr_sc"],
            "dinv_no": ci["dinv_no"],
            "a_no": ci["a_no"], "adr_no": ci["adr_no"],
        })

    kw = {}
    if TRACE:
        import os
        os.makedirs("/tmp/bass_trace", exist_ok=True)
        kw = dict(trace=True, trace_cores=[0], tmpdir="/tmp/bass_trace")
    res = run_bass_kernel_spmd(nc, in_maps, list(range(NCORES)), **kw)
    global LAST_RESULTS
    LAST_RESULTS = res
    outs = [res.results[c]["out"][:SHARD] for c in range(NCORES)]
    return np.concatenate(outs, axis=0)

